# revision 1
# baseline (speedup 1.0000x reference)
"""Multi-head attention (B=4, S=2048, D=1024, H=16) on 8 Trainium2 cores.

Sharding: core c handles batch b = c//2 and head-group g = c%2 (8 heads,
512 features). Device program is identical on all cores (SPMD); the host
feeds each core its batch's activations (pre-transposed to [D, S]) and its
head-group's weight slices, and sums the two partial output projections
per batch at the end (core g=1 gets a zero bias so the bias is added once).

Device-side layout (per core):
  qT/kT:  [512 f, S]  (f on partitions, chunked [128, 4, S])   f = 8 heads x 64
  v:      [S, 520]    (kj on partitions, per head 64 cols + a ones column)
  scores: sT[kj, qi] = k q^T tiles in PSUM -> exp -> probsT bf16 SBUF
  PV:     out_aug[65, qi] = v_aug^T @ probsT  (row 64 = softmax denominator,
          via the ones column), accumulated over kj chunks in PSUM
  divide: ls copy on ACT releases the accumulator; denominator row broadcast
          over 64 partitions (DRAM bounce DMA); approx-reciprocal on DVE;
          final multiply on GPSIMD (keeps the chain off the exp engines)
  y:      yT[j, qi] partial = woT^T @ outT (+ bias), bf16 to DRAM (the host
          upcasts and sums the two per-batch partials in fp32)

PSUM budget is static (8 banks): 3 x [128,1024] score/projection tiles (tag
"wide") + 1 x [65,1024] PV accumulator, so the whole body can also sit
inside a hardware repeat loop (`repeat` > 1, timing harness only).

Why this shape: the baseline (all exp on ACT) is ACT-bound: 33.5M score
elements per core through the scalar engine at 1 elem/lane/cycle is ~266us,
while the attention-phase PE work is ~218us. The fix is an ENGINE SPLIT of
the softmax exp: `ndve` of each window's 16 kj-chunk tiles run on DVE as a
one-instruction Schraudolph exp — i16 = trunc(s*A + B) writes the bf16 BIT
PATTERN via the truncating int16 convert (A = scale*128*log2e), ~1.8% rms,
mean ~0 — and the rest run true Exp on ACT. With ndve=6 both engines stay
under the PE rate and attention becomes PE-bound (~218us); end-to-end
rel-err goes 6.0e-3 -> 1.15e-2 (gate 2e-2). The divide chain must NOT run
on DVE or its bursts delay DVE-assigned exp tiles and stall the PE through
the 3-slot PSUM rotation (costs ~30us; measured) — hence ls-copy on ACT and
the od multiply on the otherwise-idle GPSIMD.

Attention matmuls reach only 50% array utilization (contraction/output dim
= 64 < 128), but that is irreducible here: serial matmul cost is the
moving-pass length regardless of K, PE tile_position packing does NOT run
concurrently through this codegen path (measured X/Y = 1.03), and two
concurrent matmuls must never share a PSUM bank's column space (device
wedge). fp8 DoubleRow would halve pass cost but e4m3 quantization of any
attention operand adds >=3% output error — over budget. Phases stay serial
(projections -> attention -> output projection); the first
weight/activation loads are split per contraction chunk so the first
matmul starts ~1us in.
"""

import numpy as np
import ml_dtypes

import concourse.bacc as bacc
import concourse.bass as bass
import concourse.mybir as mybir
import concourse.tile as tile

BF16 = mybir.dt.bfloat16
F32 = mybir.dt.float32
I16 = mybir.dt.int16

LOG2E = float(np.log2(np.e))
# Schraudolph bf16-bit-pattern exp via truncating int16 convert:
# i16 = trunc(score * (scale*128*log2e) + (127*128 - 7.4 + 0.5))
SCHRAUD_B = 127.0 * 128.0 - 7.4 + 0.5

B, S, D, H = 4, 2048, 1024, 16
HD = 64
N_CORES = 8
F = D // 2  # features per core (8 heads x 64)


def build_nc(s=S, d=D, f=F, num_devices=N_CORES, repeat=1, ndve=6):
    """Build the per-core Bass program. Parametrized so a small config can be
    validated in CoreSim; the shipped kernel uses the defaults."""
    hpc = f // HD          # heads per core
    dc = d // 128          # contraction chunks for projections
    fc = f // 128          # feature chunks (out partitions for q/k proj)
    jc = d // 128          # output-feature chunks for the final projection
    kc = s // 128          # kj chunks for attention
    aq = 1024 if s % 1024 == 0 else 512  # wide tile / attention qi block
    nq = s // aq
    scale = 1.0 / np.sqrt(HD)
    schraud_a = scale * 128.0 * LOG2E
    # Bresenham-spread DVE exp chunks: ndve of the kc kj-chunks per window
    dve_set = {c for c in range(kc)
               if (c * ndve) // kc != ((c + 1) * ndve) // kc}
    assert f <= 512

    nc = bacc.Bacc("TRN2", target_bir_lowering=False, debug=False,
                   num_devices=num_devices)

    xq = nc.dram_tensor("xq_t", [d, s], BF16, kind="ExternalInput").ap()
    xk = nc.dram_tensor("xk_t", [d, s], BF16, kind="ExternalInput").ap()
    xv = nc.dram_tensor("xv_t", [d, s], BF16, kind="ExternalInput").ap()
    wq = nc.dram_tensor("wq_t", [d, f], BF16, kind="ExternalInput").ap()
    wk = nc.dram_tensor("wk_t", [d, f], BF16, kind="ExternalInput").ap()
    wv = nc.dram_tensor("wv_t", [d, f], BF16, kind="ExternalInput").ap()
    wo = nc.dram_tensor("wo_t", [f, d], BF16, kind="ExternalInput").ap()
    bo = nc.dram_tensor("bo_r", [128, jc], F32, kind="ExternalInput").ap()
    y = nc.dram_tensor("y_t", [d, s], BF16, kind="ExternalOutput").ap()

    with tile.TileContext(nc) as tc:
        with (
            tc.tile_pool(name="weights", bufs=1) as wpool,
            tc.tile_pool(name="store", bufs=1) as store,
            tc.tile_pool(name="xin", bufs=2) as xpool,
            tc.tile_pool(name="probs", bufs=4) as ppool,
            tc.tile_pool(name="bcast", bufs=2) as bpool,
            tc.tile_pool(name="odiv", bufs=3) as opool,
            tc.tile_pool(name="ystage", bufs=3) as ypool,
            tc.tile_pool(name="ldram", bufs=4, space="DRAM") as dpool,
            tc.tile_pool(name="psum", bufs=3, space="PSUM") as psum,
            tc.tile_pool(name="psumo", bufs=1, space="PSUM") as psumo,
        ):
            # ---- persistent SBUF state ----
            wq_sb = wpool.tile([128, dc, f], BF16, tag="wq")
            wk_sb = wpool.tile([128, dc, f], BF16, tag="wk")
            wv_sb = wpool.tile([128, dc, f], BF16, tag="wv")
            wo_sb = wpool.tile([128, fc, d], BF16, tag="wo")
            bo_sb = wpool.tile([128, jc], F32, tag="bo")
            # wk in halves: the first projection matmul waits 512KB, not
            # the whole 1MB (per-chunk splitting measured worse in repeat
            # mode); wv ahead of wq on the gpsimd queue (v projection needs
            # it at ~27us, wq not until ~55us)
            wkr = wk.rearrange("(c p) f -> p c f", p=128)
            h2 = dc // 2
            nc.sync.dma_start(out=wk_sb[:, :h2], in_=wkr[:, :h2])
            nc.sync.dma_start(out=wk_sb[:, h2:], in_=wkr[:, h2:])
            nc.gpsimd.dma_start(out=wv_sb, in_=wv.rearrange("(c p) f -> p c f", p=128))
            nc.gpsimd.dma_start(out=wq_sb, in_=wq.rearrange("(c p) f -> p c f", p=128))
            nc.gpsimd.dma_start(out=wo_sb, in_=wo.rearrange("(c p) j -> p c j", p=128))
            nc.gpsimd.dma_start(out=bo_sb, in_=bo)

            qT_sb = store.tile([128, fc, s], BF16, tag="qT")
            kT_sb = store.tile([128, fc, s], BF16, tag="kT")
            v_sb = store.tile([128, kc, hpc * 65], BF16, tag="v")
            outT_sb = store.tile([128, fc, s], BF16, tag="outT")

            # x slices are loaded once and shared between the prefix (feature
            # chunk 0) and the deferred per-chunk projections; the pool slot
            # stays live until the last emitted consumer
            xcache = {}

            def get_x(x_dram, q):
                key = (x_dram.tensor.name, q)
                if key not in xcache:
                    xr = x_dram.rearrange("(c p) s -> p c s", p=128)
                    t = xpool.tile([128, dc, aq], BF16, tag="x")
                    h4 = max(1, dc // 4)
                    for c0 in range(0, dc, h4):
                        c1 = min(dc, c0 + h4)
                        nc.sync.dma_start(
                            out=t[:, c0:c1],
                            in_=xr[:, c0:c1, q * aq:(q + 1) * aq])
                    xcache[key] = t
                return xcache[key]

            def qk_proj_slice(x_dram, w_sb, dst, q, fis):
                """qT/kT projection for one aq slice and a list of f chunks."""
                sl = slice(q * aq, (q + 1) * aq)
                x_sb = get_x(x_dram, q)
                for fi in fis:
                    ps = psum.tile([128, aq], F32, tag="wide")
                    for qq in range(aq // 512):
                        for ci in range(dc):
                            nc.tensor.matmul(
                                ps[:, qq * 512:(qq + 1) * 512],
                                lhsT=w_sb[:, ci, fi * 128:(fi + 1) * 128],
                                rhs=x_sb[:, ci, qq * 512:(qq + 1) * 512],
                                start=(ci == 0), stop=(ci == dc - 1))
                    nc.vector.tensor_copy(out=dst[:, fi, sl], in_=ps)

            def v_proj_slice(q):
                """v projection (with ones columns interleaved) for one slice."""
                xr = xv.rearrange("(c p) s -> p c s", p=128)
                nchunks = aq // 128   # kj chunks per aq slice
                cpt = aq // f         # kj chunks packed per psum tile
                sl = slice(q * aq, (q + 1) * aq)
                x_sb = xpool.tile([128, dc, aq], BF16, tag="x")
                nc.sync.dma_start(out=x_sb, in_=xr[:, :, sl])
                for t in range(nchunks // cpt):
                    ps = psum.tile([128, aq], F32, tag="wide")
                    for k4 in range(cpt):
                        ck = t * cpt + k4
                        for ci in range(dc):
                            nc.tensor.matmul(
                                ps[:, k4 * f:(k4 + 1) * f],
                                lhsT=x_sb[:, ci, ck * 128:(ck + 1) * 128],
                                rhs=wv_sb[:, ci],
                                start=(ci == 0), stop=(ci == dc - 1))
                        cix = q * nchunks + ck
                        nc.vector.tensor_copy(
                            out=v_sb[:, cix].rearrange(
                                "p (h x) -> p h x", x=65)[:, :, 0:64],
                            in_=ps[:, k4 * f:(k4 + 1) * f].rearrange(
                                "p (h x) -> p h x", x=64))

            def attention(h, qb, queue=()):
                hp = (h % 2) * 64
                hc = h // 2
                kTh = kT_sb[hp:hp + 64, hc]
                qTh = qT_sb[hp:hp + 64, hc]
                oaug = psumo.tile([65, aq], F32, tag="oaug")
                prs = {}

                def pv(c):
                    # PV consumption pipelined 2 chunks behind the exp so the
                    # PE never waits exp latency (~1.0-1.2us > its 853ns/chunk
                    # rate): between exp(c) issue and PV(c) the PE streams
                    # sc(c+1), PV(c-1), sc(c+2) — ~1.7us of its own work
                    pr = prs.pop(c)
                    for qq in range(aq // 512):
                        nc.tensor.matmul(
                            oaug[:, qq * 512:(qq + 1) * 512],
                            lhsT=v_sb[:, c, h * 65:(h + 1) * 65],
                            rhs=pr[:, qq * 512:(qq + 1) * 512],
                            start=(c == 0), stop=(c == kc - 1),
                            skip_group_check=True)

                for c in range(kc):
                    if queue and c in (kc // 3, (2 * kc) // 3):
                        queue.pop(0)()
                    sc = psum.tile([128, aq], F32, tag="wide")
                    for qq in range(aq // 512):
                        nc.tensor.matmul(
                            sc[:, qq * 512:(qq + 1) * 512],
                            lhsT=kTh[:, c * 128:(c + 1) * 128],
                            rhs=qTh[:, qb * aq + qq * 512:
                                    qb * aq + (qq + 1) * 512],
                            start=True, stop=True)
                    pr = ppool.tile([128, aq], BF16, tag="pr")
                    # exp split: DVE runs a one-instruction Schraudolph exp
                    # (bf16 bits via truncating int16 convert, ~1.8% rms) on
                    # `ndve` of the kc chunks; ACT runs true Exp on the rest.
                    # This removes the ACT bottleneck (ACT-only attention is
                    # ~266us vs ~218us of PE work).
                    if c in dve_set:
                        nc.vector.tensor_scalar(
                            out=pr.bitcast(I16), in0=sc,
                            scalar1=float(schraud_a),
                            scalar2=float(SCHRAUD_B),
                            op0=mybir.AluOpType.mult,
                            op1=mybir.AluOpType.add)
                    else:
                        nc.scalar.activation(
                            out=pr, in_=sc,
                            func=mybir.ActivationFunctionType.Exp,
                            scale=float(scale))
                    prs[c] = pr
                    if c >= 2:
                        pv(c - 2)
                pv(kc - 2)
                pv(kc - 1)
                # stage the denominator row to SBUF (DMA cannot read PSUM),
                # broadcast it over 64 partitions via a DRAM bounce, divide
                ls = bpool.tile([65, aq], F32, tag="ls")
                # ls copy on ACT, od mul on GPSIMD: keeps the divide chain
                # off DVE so DVE-assigned exp tiles never back up behind it
                nc.scalar.copy(out=ls, in_=oaug)
                ld = dpool.tile([1, aq], F32, tag="ld")
                nc.sync.dma_start(out=ld, in_=ls[64:65])
                bc = bpool.tile([64, aq], F32, tag="bc")
                nc.gpsimd.dma_start(out=bc, in_=ld.to_broadcast([64, aq]))
                nc.vector.reciprocal_approx_fast(out=bc, in_=bc)
                od = opool.tile([64, aq], BF16, tag="od")
                nc.gpsimd.tensor_mul(out=od, in0=ls[0:64], in1=bc)
                nc.sync.dma_start(
                    out=outT_sb[hp:hp + 64, hc, qb * aq:(qb + 1) * aq],
                    in_=od)

            def wo_proj_group(q, j):
                """output projection yT[j chunk, aq slice] = woT^T @ outT + bias"""
                sl = slice(q * aq, (q + 1) * aq)
                ps = psum.tile([128, aq], F32, tag="wide")
                for qq in range(aq // 512):
                    for fi in range(fc):
                        nc.tensor.matmul(
                            ps[:, qq * 512:(qq + 1) * 512],
                            lhsT=wo_sb[:, fi, j * 128:(j + 1) * 128],
                            rhs=outT_sb[:, fi,
                                        q * aq + qq * 512:q * aq + (qq + 1) * 512],
                            start=(fi == 0), stop=(fi == fc - 1))
                ys = ypool.tile([128, aq], BF16, tag="ys")
                nc.scalar.activation(
                    out=ys, in_=ps,
                    func=mybir.ActivationFunctionType.Identity,
                    bias=bo_sb[:, j:j + 1], scale=1.0)
                nc.sync.dma_start(out=y[j * 128:(j + 1) * 128, sl], in_=ys)

            def body(_iv=None):
                # ones columns of v_aug (one strided memset per kj chunk)
                for c in range(kc):
                    nc.vector.memset(
                        v_sb[:, c].rearrange("p (h x) -> p h x", x=65)[:, :, 64:65],
                        1.0)

                # serial phases: projections, attention, output projection
                for q in range(nq):
                    qk_proj_slice(xk, wk_sb, kT_sb, q, range(fc))
                for q in range(nq):
                    qk_proj_slice(xq, wq_sb, qT_sb, q, range(fc))
                for q in range(nq):
                    v_proj_slice(q)
                for h in range(hpc):
                    for qb in range(nq):
                        attention(h, qb)
                for q in range(nq):
                    for j in range(jc):
                        wo_proj_group(q, j)

            if repeat == 1:
                body()
            else:
                with tc.For_i(0, repeat, 1) as iv:
                    body(iv)

    nc.compile()
    return nc


def make_core_inputs(query, key, value, wq, wk, wv, wo, bo):
    """Host-side sharding: per-core input dicts (bf16 casts + transposes)."""
    bf = ml_dtypes.bfloat16
    query, key, value = (np.asarray(t, np.float32) for t in (query, key, value))
    wq, wk, wv, wo, bo = (np.asarray(t, np.float32) for t in (wq, wk, wv, wo, bo))
    ins = []
    for c in range(N_CORES):
        b, g = c // 2, c % 2
        fs = slice(g * F, (g + 1) * F)
        ins.append({
            "xq_t": np.ascontiguousarray(query[b].astype(bf).T),
            "xk_t": np.ascontiguousarray(key[b].astype(bf).T),
            "xv_t": np.ascontiguousarray(value[b].astype(bf).T),
            "wq_t": np.ascontiguousarray(wq[fs, :].T.astype(bf)),
            "wk_t": np.ascontiguousarray(wk[fs, :].T.astype(bf)),
            "wv_t": np.ascontiguousarray(wv[fs, :].T.astype(bf)),
            "wo_t": np.ascontiguousarray(wo[:, fs].T.astype(bf)),
            "bo_r": (bo.reshape(D // 128, 128).T.astype(np.float32)
                     if g == 0 else np.zeros((128, D // 128), np.float32)),
        })
    return ins


_NC_CACHE = None


def kernel(query, key, value, wq, wk, wv, wo, bo):
    global _NC_CACHE
    from concourse.bass_utils import run_bass_kernel_spmd

    if _NC_CACHE is None:
        _NC_CACHE = build_nc()
    ins = make_core_inputs(query, key, value, wq, wk, wv, wo, bo)
    res = run_bass_kernel_spmd(_NC_CACHE, ins, list(range(N_CORES)))
    out = np.empty((B, S, D), np.float32)
    for b in range(B):
        out[b] = (res.results[2 * b]["y_t"].astype(np.float32)
                  + res.results[2 * b + 1]["y_t"].astype(np.float32)).T
    return out



# revision 8
# speedup vs baseline: 1.0336x; 1.0336x over previous
"""Multi-head attention (B=4, S=2048, D=1024, H=16) on 8 Trainium2 cores.

Sharding: core c handles batch b = c//2 and head-group g = c%2 (8 heads,
512 features). SPMD device program; host feeds per-core slices (activations
pre-transposed to [D, S]) and sums the two partial output projections per
batch (core g=1 gets a zero bias).

v3 = v1 + row-packed score matmuls + injected projections.

PE tile_position packing DOES run concurrently through this codegen path
(microbenched here: K=64 M=128 N=512 pairs at (0,0)/(64,0) hit 105 ns/MM
vs 249 for a K=128 reference and 405 for unpacked K=64 at one position;
numerics exact). Head pair (2i, 2i+1) lives in the partition halves of
kT/qT chunk i, so per kj chunk and qi half TWO K=64 score matmuls (one
per head) run concurrently in the upper/lower 64 PE rows -> 2x score
throughput, and the whole head pair's probs are produced together.

What was tried and REVERTED (v2, documented so it is not retried): PV
col-packing ((0,0)/(0,64) M=64 into one bank) and a 4x M=1 ones-tile
denominator quad are individually correct in isolation, but in the mixed
instruction stream concurrent partial-tile matmuls clobber each other's
in-flight weights (PV/projection results hit 1e28 garbage -> NaN), and
one timing variant wedged the device (NRT_EXEC_UNIT_UNRECOVERABLE) --
consistent with v1's "concurrent matmuls must never share a PSUM bank's
column space" note. PV therefore stays in the v1 form: ones-augmented
v (M=65, all at (0,0), K=128) accumulating into a [65, 1024] bank pair
per head -- every PV matmul conflicts with the whole array, acting as a
serializing anchor between packed score pairs and injected projections.
The attention phase is exp-throughput bound anyway (ACT 1.2 GHz + DVE
0.96 GHz ~ 276 Gel/s on 262K score els per chunk vs ~1.3 us of PE work),
so PV packing bought no wall time.

softmax: exp split ACT (true Exp) / DVE (one-instruction Schraudolph
bf16-bit-pattern exp via truncating int16 convert, ~1.8% rms) by a
Bresenham fraction act_frac. Projections and the output projection are
emitted as ~8-matmul granules injected between attention chunks/blocks
so the PE keeps busy while the exp engines drain. GPSIMD/Pool cannot
read PSUM (BIR verifier), so Pool only does the divide multiply and the
broadcast DMAs, exactly as v1.

PSUM (8 banks): 4 rotating [128,512] score/projection tiles + 2x
[65,1024] PV accumulators (one per head of the pair, 2 banks each).
"""

import numpy as np
import ml_dtypes

import concourse.bacc as bacc
import concourse.bass as bass
import concourse.mybir as mybir
import concourse.tile as tile

BF16 = mybir.dt.bfloat16
F32 = mybir.dt.float32
I16 = mybir.dt.int16

LOG2E = float(np.log2(np.e))
# Schraudolph bf16-bit-pattern exp via truncating int16 convert:
# i16 = trunc(score * (scale*128*log2e) + (127*128 - 7.4 + 0.5))
SCHRAUD_B = 127.0 * 128.0 - 7.4 + 0.5

B, S, D, H = 4, 2048, 1024, 16
HD = 64
N_CORES = 8
F = D // 2  # features per core (8 heads x 64)


def build_nc(s=S, d=D, f=F, num_devices=N_CORES, repeat=1, act_frac=0.55):
    """Build the per-core Bass program. s=1024 gives a single-qb config that
    CoreSim can check; the shipped kernel uses the defaults."""
    hpc = f // HD          # heads per core (8)
    npair = hpc // 2       # head pairs (4)
    dc = d // 128          # contraction chunks for projections (8)
    fc = f // 128          # feature chunks of qT/kT (4)
    jc = d // 128          # output-feature chunks for the final projection
    kc = s // 128          # kj chunks (16)
    aq = 1024              # qi block (2 x 512 halves)
    nq = s // aq
    lag = 2                # PV consumption lag behind exp, in chunks
    scale = 1.0 / np.sqrt(HD)
    schraud_a = scale * 128.0 * LOG2E
    assert f == 512 and kc >= lag + 2 and s % aq == 0

    nc = bacc.Bacc("TRN2", target_bir_lowering=False, debug=False,
                   num_devices=num_devices)

    xq = nc.dram_tensor("xq_t", [d, s], BF16, kind="ExternalInput").ap()
    xk = nc.dram_tensor("xk_t", [d, s], BF16, kind="ExternalInput").ap()
    xv = nc.dram_tensor("xv_t", [d, s], BF16, kind="ExternalInput").ap()
    wq = nc.dram_tensor("wq_t", [d, f], BF16, kind="ExternalInput").ap()
    wk = nc.dram_tensor("wk_t", [d, f], BF16, kind="ExternalInput").ap()
    wv = nc.dram_tensor("wv_t", [d, f], BF16, kind="ExternalInput").ap()
    wo = nc.dram_tensor("wo_t", [f, d], BF16, kind="ExternalInput").ap()
    bo = nc.dram_tensor("bo_r", [128, jc], F32, kind="ExternalInput").ap()
    y = nc.dram_tensor("y_t", [d, s], BF16, kind="ExternalOutput").ap()

    with tile.TileContext(nc) as tc:
        with (
            tc.tile_pool(name="weights", bufs=1) as wpool,
            tc.tile_pool(name="store", bufs=1) as store,
            tc.tile_pool(name="xin", bufs=3) as xpool,
            tc.tile_pool(name="xvin", bufs=2) as xvpool,
            tc.tile_pool(name="probs", bufs=11) as ppool,
            tc.tile_pool(name="bcast", bufs=2) as bpool,
            tc.tile_pool(name="odiv", bufs=2) as opool,
            tc.tile_pool(name="ystage", bufs=3) as ypool,
            tc.tile_pool(name="ldram", bufs=4, space="DRAM") as dpool,
            tc.tile_pool(name="psum", bufs=4, space="PSUM") as pspool,
            tc.tile_pool(name="psumo", bufs=2, space="PSUM") as pvpool,
        ):
            # ---- persistent SBUF state ----
            wq_sb = wpool.tile([128, dc, f], BF16, tag="wq")
            wk_sb = wpool.tile([128, dc, f], BF16, tag="wk")
            wv_sb = wpool.tile([128, dc, f], BF16, tag="wv")
            wo_sb = wpool.tile([128, fc, d], BF16, tag="wo")
            bo_sb = wpool.tile([128, jc], F32, tag="bo")
            # wk halves first on the sync queue (first projection), the rest
            # on the gpsimd queue in the order the schedule needs them
            wkr = wk.rearrange("(c p) f -> p c f", p=128)
            h2 = dc // 2
            nc.sync.dma_start(out=wk_sb[:, :h2], in_=wkr[:, :h2])
            nc.sync.dma_start(out=wk_sb[:, h2:], in_=wkr[:, h2:])
            nc.gpsimd.dma_start(out=wq_sb, in_=wq.rearrange("(c p) f -> p c f", p=128))
            nc.gpsimd.dma_start(out=wv_sb, in_=wv.rearrange("(c p) f -> p c f", p=128))
            nc.gpsimd.dma_start(out=wo_sb, in_=wo.rearrange("(c p) j -> p c j", p=128))
            nc.gpsimd.dma_start(out=bo_sb, in_=bo)

            qT_sb = store.tile([128, fc, s], BF16, tag="qT")
            kT_sb = store.tile([128, fc, s], BF16, tag="kT")
            v_sb = store.tile([128, kc, hpc * 65], BF16, tag="v")
            outT_sb = store.tile([128, fc, s], BF16, tag="outT")

            xcache = {}

            def get_xv(h):
                # v-projection x slices at 512-col granularity (8KB/part)
                key = ("xv_h", h)
                if key not in xcache:
                    xr = xv.rearrange("(c p) s -> p c s", p=128)
                    t = xvpool.tile([128, dc, 512], BF16, tag="xv")
                    nc.sync.dma_start(
                        out=t, in_=xr[:, :, h * 512:(h + 1) * 512])
                    xcache[key] = t
                return xcache[key]

            def get_x(x_dram, q, pool):
                key = (x_dram.tensor.name, q)
                if key not in xcache:
                    xr = x_dram.rearrange("(c p) s -> p c s", p=128)
                    t = pool.tile([128, dc, aq], BF16, tag="x")
                    h4 = max(1, dc // 4)
                    for c0 in range(0, dc, h4):
                        c1 = min(dc, c0 + h4)
                        nc.sync.dma_start(
                            out=t[:, c0:c1],
                            in_=xr[:, c0:c1, q * aq:(q + 1) * aq])
                    xcache[key] = t
                return xcache[key]

            # ---- projection granules (~8 matmuls each, injectable) ----
            def qk_granule(x_dram, w_sb, dst, fi, q, qq):
                x_sb = get_x(x_dram, q, xpool)
                ps = pspool.tile([128, 512], F32, tag="wide")
                col = q * aq + qq * 512
                for ci in range(dc):
                    nc.tensor.matmul(
                        ps, lhsT=w_sb[:, ci, fi * 128:(fi + 1) * 128],
                        rhs=x_sb[:, ci, qq * 512:(qq + 1) * 512],
                        start=(ci == 0), stop=(ci == dc - 1))
                nc.vector.tensor_copy(out=dst[:, fi, col:col + 512], in_=ps)

            def v_granule(c):
                # one kj chunk of the v projection, with the 65th (ones)
                # column interleave of the v1 layout
                x_sb = get_xv(c // 4)
                ps = pspool.tile([128, 512], F32, tag="wide")
                lc = (c % 4) * 128
                for ci in range(dc):
                    nc.tensor.matmul(
                        ps, lhsT=x_sb[:, ci, lc:lc + 128], rhs=wv_sb[:, ci],
                        start=(ci == 0), stop=(ci == dc - 1))
                nc.vector.tensor_copy(
                    out=v_sb[:, c].rearrange("p (h x) -> p h x", x=65)[:, :, 0:64],
                    in_=ps.rearrange("p (h x) -> p h x", x=64))

            def wo_granule(q, j, qq):
                ps = pspool.tile([128, 512], F32, tag="wide")
                col = q * aq + qq * 512
                for fi in range(fc):
                    nc.tensor.matmul(
                        ps, lhsT=wo_sb[:, fi, j * 128:(j + 1) * 128],
                        rhs=outT_sb[:, fi, col:col + 512],
                        start=(fi == 0), stop=(fi == fc - 1))
                ys = ypool.tile([128, 512], BF16, tag="ys")
                nc.scalar.activation(
                    out=ys, in_=ps,
                    func=mybir.ActivationFunctionType.Identity,
                    bias=bo_sb[:, j:j + 1], scale=1.0)
                nc.sync.dma_start(out=y[j * 128:(j + 1) * 128, col:col + 512],
                                  in_=ys)

            expi = [0]

            def attention_block(pair, qb, mid):
                qoff = qb * aq
                psPV0 = pvpool.tile([65, aq], F32, tag="pv")
                psPV1 = pvpool.tile([65, aq], F32, tag="pv")
                psPV = [psPV0, psPV1]
                prs = {}

                def expand(c, qq, par, ps):
                    # ACT true exp / DVE Schraudolph, Bresenham by act_frac
                    pr = ppool.tile([128, 512], BF16, tag="pr")
                    i = expi[0]
                    expi[0] += 1
                    on_act = int(i * act_frac) != int((i + 1) * act_frac)
                    if on_act:
                        nc.scalar.activation(
                            out=pr, in_=ps,
                            func=mybir.ActivationFunctionType.Exp,
                            scale=float(scale))
                    else:
                        nc.vector.tensor_scalar(
                            out=pr.bitcast(I16), in0=ps,
                            scalar1=float(schraud_a),
                            scalar2=float(SCHRAUD_B),
                            op0=mybir.AluOpType.mult,
                            op1=mybir.AluOpType.add)
                    prs[(par, c, qq)] = pr

                def pv(c):
                    # v1-style PV: ones-augmented v, M=65, all at (0,0);
                    # row 64 accumulates the softmax denominator
                    for par in (0, 1):
                        vh = v_sb[:, c, (2 * pair + par) * 65:
                                  (2 * pair + par) * 65 + 65]
                        for qq in (0, 1):
                            nc.tensor.matmul(
                                psPV[par][:, qq * 512:(qq + 1) * 512],
                                lhsT=vh, rhs=prs.pop((par, c, qq)),
                                start=(c == 0), stop=(c == kc - 1),
                                skip_group_check=True)

                for c in range(kc):
                    for item in mid.get(c, []):
                        item()
                    for qq in (0, 1):
                        pse = pspool.tile([128, 512], F32, tag="wide")
                        pso = pspool.tile([128, 512], F32, tag="wide")
                        nc.tensor.matmul(
                            pse,
                            lhsT=kT_sb[0:64, pair, c * 128:(c + 1) * 128],
                            rhs=qT_sb[0:64, pair,
                                      qoff + qq * 512:qoff + (qq + 1) * 512],
                            start=True, stop=True)
                        nc.tensor.matmul(
                            pso,
                            lhsT=kT_sb[64:128, pair, c * 128:(c + 1) * 128],
                            rhs=qT_sb[64:128, pair,
                                      qoff + qq * 512:qoff + (qq + 1) * 512],
                            start=True, stop=True)
                        expand(c, qq, 0, pse)
                        expand(c, qq, 1, pso)
                    if c >= lag:
                        pv(c - lag)
                for c in range(kc - lag, kc):
                    pv(c)

                # divide + store (v1 chain): ls copy on ACT releases the
                # accumulator; denominator row broadcast over 64 partitions
                # via a DRAM bounce; approx-reciprocal on DVE; multiply on
                # Pool; DMA to outT
                for par in (0, 1):
                    hp = par * 64
                    ls = bpool.tile([65, aq], F32, tag="ls")
                    nc.scalar.copy(out=ls, in_=psPV[par])
                    ld = dpool.tile([1, aq], F32, tag="ld")
                    nc.sync.dma_start(out=ld, in_=ls[64:65])
                    bc = bpool.tile([64, aq], F32, tag="bc")
                    nc.gpsimd.dma_start(out=bc, in_=ld.to_broadcast([64, aq]))
                    nc.vector.reciprocal_approx_fast(out=bc, in_=bc)
                    od = opool.tile([64, aq], BF16, tag="od")
                    nc.gpsimd.tensor_mul(out=od, in0=ls[0:64], in1=bc)
                    nc.sync.dma_start(
                        out=outT_sb[hp:hp + 64, pair, qoff:qoff + aq],
                        in_=od)

            def body(_iv=None):
                xcache.clear()
                expi[0] = 0
                # ones columns of v_aug (one strided memset per kj chunk)
                for c in range(kc):
                    nc.vector.memset(
                        v_sb[:, c].rearrange("p (h x) -> p h x", x=65)[:, :, 64:65],
                        1.0)

                KT = lambda fi, q, qq: (lambda: qk_granule(xk, wk_sb, kT_sb, fi, q, qq))
                QT = lambda fi, q, qq: (lambda: qk_granule(xq, wq_sb, qT_sb, fi, q, qq))
                V = lambda c: (lambda: v_granule(c))
                WO = lambda q, j, qq: (lambda: wo_granule(q, j, qq))
                PFX = lambda x_dram, q, pool: (lambda: get_x(x_dram, q, pool))
                PFV = lambda h: (lambda: get_xv(h))

                # prologue: first head-pair's kT/qT for qb0 + first v chunks
                PFV(0)()
                for it in (KT(0, 0, 0), KT(0, 0, 1), QT(0, 0, 0),
                           QT(0, 0, 1), V(0), V(1)):
                    it()

                # per-block mid-chunk injection schedules
                def b0_mid():
                    mid = {0: ([PFX(xk, 1, xpool)] if nq > 1 else [])
                              + [PFV(1), V(2)]}
                    vc = 3
                    for c in range(1, kc - 2):
                        mid[c] = []
                        if nq > 1 and c == 5:
                            mid[c].append(KT(0, 1, 0))
                        if nq > 1 and c == 6:
                            mid[c].append(KT(0, 1, 1))
                        if c == 3 and kc > 8:
                            mid[c].append(PFV(2))
                        if c == 7 and kc > 12:
                            mid[c].append(PFV(3))
                        if vc < kc:
                            mid[c].append(V(vc))
                            vc += 1
                    return mid

                def kt_mid(fi):
                    if nq == 1:
                        return {}
                    return {2: [KT(fi, 1, 0)], 4: [KT(fi, 1, 1)]}

                def wo_mid(js):
                    return {2: [WO(0, js[0], 0)], 5: [WO(0, js[0], 1)],
                            8: [WO(0, js[1], 0)], 11: [WO(0, js[1], 1)]}

                attention_block(0, 0, b0_mid())
                for pair in range(1, npair):
                    for it in (KT(pair, 0, 0), KT(pair, 0, 1),
                               QT(pair, 0, 0), QT(pair, 0, 1)):
                        it()
                    mid = kt_mid(pair)
                    if pair == npair - 1 and nq > 1:
                        mid.setdefault(6, []).append(PFX(xq, 1, xpool))
                    attention_block(pair, 0, mid)

                if nq > 1:
                    for pair in range(npair):
                        QT(pair, 1, 0)()
                        QT(pair, 1, 1)()
                        attention_block(pair, 1,
                                        wo_mid((2 * pair, 2 * pair + 1)))
                    for j in range(jc):
                        for qq in (0, 1):
                            wo_granule(1, j, qq)
                else:
                    for j in range(jc):
                        for qq in (0, 1):
                            wo_granule(0, j, qq)

            if repeat == 1:
                body()
            else:
                with tc.For_i(0, repeat, 1) as iv:
                    body(iv)

    nc.compile()
    return nc


def make_core_inputs(query, key, value, wq, wk, wv, wo, bo):
    """Host-side sharding: per-core input dicts (bf16 casts + transposes)."""
    bf = ml_dtypes.bfloat16
    query, key, value = (np.asarray(t, np.float32) for t in (query, key, value))
    wq, wk, wv, wo, bo = (np.asarray(t, np.float32) for t in (wq, wk, wv, wo, bo))
    ins = []
    for c in range(N_CORES):
        b, g = c // 2, c % 2
        fs = slice(g * F, (g + 1) * F)
        ins.append({
            "xq_t": np.ascontiguousarray(query[b].astype(bf).T),
            "xk_t": np.ascontiguousarray(key[b].astype(bf).T),
            "xv_t": np.ascontiguousarray(value[b].astype(bf).T),
            "wq_t": np.ascontiguousarray(wq[fs, :].T.astype(bf)),
            "wk_t": np.ascontiguousarray(wk[fs, :].T.astype(bf)),
            "wv_t": np.ascontiguousarray(wv[fs, :].T.astype(bf)),
            "wo_t": np.ascontiguousarray(wo[:, fs].T.astype(bf)),
            "bo_r": (bo.reshape(D // 128, 128).T.astype(np.float32)
                     if g == 0 else np.zeros((128, D // 128), np.float32)),
        })
    return ins


_NC_CACHE = None


def kernel(query, key, value, wq, wk, wv, wo, bo):
    global _NC_CACHE
    from concourse.bass_utils import run_bass_kernel_spmd

    if _NC_CACHE is None:
        _NC_CACHE = build_nc()
    ins = make_core_inputs(query, key, value, wq, wk, wv, wo, bo)
    res = run_bass_kernel_spmd(_NC_CACHE, ins, list(range(N_CORES)))
    out = np.empty((B, S, D), np.float32)
    for b in range(B):
        out[b] = (res.results[2 * b]["y_t"].astype(np.float32)
                  + res.results[2 * b + 1]["y_t"].astype(np.float32)).T
    return out
